# revision 35
# baseline (speedup 1.0000x reference)
"""Trainium2 Bass kernel for 12-head attention (B=4, S=2048, D=768) on 8 cores.

Sharding: core c handles batch b=c//2, query half q0=(c%2)*1024. Each core
receives its batch's tokens rotated so its own queries are tokens 0..1023
(attention is permutation-invariant over keys, so K/V over rotated tokens
give identical output). No collectives needed.

Device algorithm (per core), everything in transposed layouts so softmax
needs no on-chip transposes:
  xT   = x^T                 [768, 2048]   (PE transpose of x tiles)
  qT   = (Wq*scale)^T @ xT   [768, 1024]   (own queries only)
  kT   = Wk^T @ xT           [768, 2048]
  V    = x @ Wv              [2048, 12*(64+1)]  (per-head 64 cols + ones col)
  per head h:
    pT[k, q]   = exp(kT_h[:,k]^T qT_h)     (no max subtraction: |scores|<~8)
    po[0:65,q] = [V_h | 1]^T @ pT          (rows 0..63 = outT, row 64 = sum)
    outT_h     = po[0:64] / po[64]         (gpsimd partition_broadcast + mul)
  y = outT^T @ Wproj + bias
All matmul operands are float32r (full fp32 storage, PE rounds to reduced
mantissa; ~1e-4 rel err, 4x faster than true fp32 matmul).
"""

import numpy as np

import concourse.bass as bass
import concourse.mybir as mybir
import concourse.tile as tile
from concourse.bass_utils import run_bass_kernel_spmd
from concourse.masks import make_identity

HEADS = 12
DIM = 768
HEAD_DIM = 64
SCALE = HEAD_DIM ** -0.5
B = 4
S = 2048
SQ = 1024          # queries per core
NCORES = 8
DC = DIM // 128    # 6 contraction chunks
TT = S // 128      # 16 token tiles
VH = HEAD_DIM + 1  # 65: per-head V cols + ones col

F32 = mybir.dt.float32
F32R = mybir.dt.float32r
BF16 = mybir.dt.bfloat16
# matmul operand dtype: float32r (safe, ~2 cyc/row) or bfloat16 (1 cyc/row)
import os
MM_DT = BF16 if os.environ.get("KMM_DT", "f32r") == "bf16" else F32R


def _cap_sync_waits(nc, max_waits=1):
    """Workaround for this walrus build's 'Too many sync wait commands':
    hoist excess per-instruction sem waits onto standalone EventSemaphore
    instructions inserted just before, on the same engine."""
    n = 0
    for fn in nc.m.functions:
        for bb in fn.blocks:
            out = []
            for inst in bb.instructions:
                si = inst.sync_info
                if si is not None and si.on_wait and len(si.on_wait) > max_waits:
                    waits = list(si.on_wait)
                    hoist, keep = waits[:-max_waits], waits[-max_waits:]
                    for w in hoist:
                        ev = mybir.InstEventSemaphore(
                            name=nc.get_next_instruction_name(), ins=[], outs=[])
                        ev.engine = inst.engine
                        ev.sync_info = mybir.SyncInfo(on_wait=[w], on_update=[])
                        out.append(ev)
                        n += 1
                    del si.on_wait[:]
                    for w in keep:
                        si.on_wait.append(w)
                out.append(inst)
            bb.instructions[:] = out
    return n




def _load_w(nc, pool, view, prefix):
    """DMA 6 [128,768] f32 weight chunks and DVE-cast them to MM_DT."""
    out = []
    for i in range(DC):
        stage = pool.tile([128, DIM], F32, tag="wstage", name=f"{prefix}s{i}",
                          bufs=2)
        nc.sync.dma_start(out=stage[:], in_=view[i])
        wt = pool.tile([128, DIM], MM_DT, tag="w", name=f"{prefix}{i}")
        nc.vector.tensor_copy(wt[:], stage[:])
        out.append(wt)
    return out

def _build_program():
    nc = bass.Bass()
    xb_in = nc.declare_dram_parameter("xb", [S, DIM], F32, isOutput=False)
    wq_in = nc.declare_dram_parameter("wq", [DIM, DIM], F32, isOutput=False)
    wk_in = nc.declare_dram_parameter("wk", [DIM, DIM], F32, isOutput=False)
    wv_in = nc.declare_dram_parameter("wv", [DIM, DIM], F32, isOutput=False)
    wp_in = nc.declare_dram_parameter("wp", [DIM, DIM], F32, isOutput=False)
    bp_in = nc.declare_dram_parameter("bp", [DIM], F32, isOutput=False)
    es_in = nc.declare_dram_parameter("es", [1, 512], F32, isOutput=False)
    y_out = nc.declare_dram_parameter("y", [SQ, DIM], F32, isOutput=True)

    xb_v = xb_in[:].rearrange("(g p) d -> p g d", p=128)   # [128, 16, 768]
    y_v = y_out[:].rearrange("(g p) d -> p g d", p=128)    # [128, 8, 768]
    w_views = {
        "wq": wq_in[:].rearrange("(c p) d -> c p d", p=128),  # [6, 128, 768]
        "wk": wk_in[:].rearrange("(c p) d -> c p d", p=128),
        "wv": wv_in[:].rearrange("(c p) d -> c p d", p=128),
    }

    with tile.TileContext(nc) as tc, \
         nc.allow_low_precision(reason="float32r matmul operands; all PSUM "
                                "accumulation stays fp32"):
        # ---- persistent tensors (live across qkv + attention) ----
        with tc.tile_pool(name="persist", bufs=1) as pp:
            kT = pp.tile([128, DC, S], MM_DT, tag="kT")       # 48 KB/part
            qT = pp.tile([128, DC, SQ], MM_DT, tag="qT")      # 24 KB/part
            v = pp.tile([128, TT, HEADS * VH], MM_DT, tag="v")  # 48.75 KB/part
            # constants: memset/affine_select can't encode f32r directly,
            # so build in f32 and round-copy via DVE.
            identf = pp.tile([128, 128], F32, tag="identf")
            make_identity(nc, identf[:])

            onesf = pp.tile([128, 64], F32, tag="onesf")
            nc.vector.memset(onesf[:], 1.0)
            # ones columns of V (col 64 of every (tile, head) group)
            v4 = v[:].rearrange("p g (h c) -> p g h c", c=VH)
            nc.vector.tensor_copy(
                v4[:, :, :, 64], onesf[:, 0:1].to_broadcast((128, TT, HEADS)))

            # ================= phase 1: xT + QKV =================
            with tc.tile_pool(name="qkv_sb", bufs=1) as qsb, \
                 tc.tile_pool(name="xnat", bufs=2) as xnp, \
                 tc.tile_pool(name="wpool", bufs=6) as wp, \
                 tc.tile_pool(name="ps_t", bufs=2, space="PSUM") as pst, \
                 tc.tile_pool(name="ps_q", bufs=3, space="PSUM") as psq:
                xT = qsb.tile([128, DC, S], MM_DT, tag="xT")  # 48 KB/part

                # transpose x into xT, 2 token-tiles per step. In bf16 mode,
                # cast during the (SWDGE) DMA so transposes run at 1 cyc/row.
                if MM_DT == BF16:
                    identt = qsb.tile([128, 128], MM_DT, tag="identt")
                    nc.vector.tensor_copy(identt[:], identf[:])
                else:
                    identt = identf
                xn_dt = MM_DT if MM_DT == BF16 else F32
                for g0 in range(0, TT, 2):
                    xn = xnp.tile([128, 2, DIM], xn_dt, tag="xn")
                    if MM_DT == BF16:
                        nc.gpsimd.dma_start(out=xn[:], in_=xb_v[:, g0:g0 + 2, :])
                    else:
                        nc.sync.dma_start(out=xn[:], in_=xb_v[:, g0:g0 + 2, :])
                    for dc in range(DC):
                        ps = pst.tile([128, 256], xn_dt, tag="pst")
                        for a in range(2):
                            nc.tensor.transpose(
                                ps[:, a * 128:(a + 1) * 128],
                                xn[:, a, dc * 128:(dc + 1) * 128],
                                identt[:])
                        nc.vector.tensor_copy(
                            xT[:, dc, g0 * 128:(g0 + 2) * 128], ps[:])

                # qT = (Wq*scale)^T @ xT  (queries = tokens 0..1023)
                wq_t = _load_w(nc, wp, w_views["wq"], "wq")
                for m in range(DC):
                    for n in range(SQ // 512):
                        ps = psq.tile([128, 512], F32, tag="psq")
                        for dc in range(DC):
                            nc.tensor.matmul(
                                ps[:],
                                wq_t[dc][:, m * 128:(m + 1) * 128],
                                xT[:, dc, n * 512:(n + 1) * 512],
                                start=(dc == 0), stop=(dc == DC - 1))
                        nc.vector.tensor_copy(qT[:, m, n * 512:(n + 1) * 512], ps[:])

                # kT = Wk^T @ xT (all tokens)
                wk_t = _load_w(nc, wp, w_views["wk"], "wk")
                for m in range(DC):
                    for n in range(S // 512):
                        ps = psq.tile([128, 512], F32, tag="psq")
                        for dc in range(DC):
                            nc.tensor.matmul(
                                ps[:],
                                wk_t[dc][:, m * 128:(m + 1) * 128],
                                xT[:, dc, n * 512:(n + 1) * 512],
                                start=(dc == 0), stop=(dc == DC - 1))
                        nc.vector.tensor_copy(kT[:, m, n * 512:(n + 1) * 512], ps[:])

                # V = x @ Wv (natural layout, strided into 65-col head groups)
                wv_t = _load_w(nc, wp, w_views["wv"], "wv")
                for g in range(TT):
                    for n2 in range(2):
                        ps = psq.tile([128, 512], F32, tag="psq")
                        for dc in range(DC):
                            nc.tensor.matmul(
                                ps[:, :384],
                                xT[:, dc, g * 128:(g + 1) * 128],
                                wv_t[dc][:, n2 * 384:(n2 + 1) * 384],
                                start=(dc == 0), stop=(dc == DC - 1))
                        nc.vector.tensor_copy(
                            v4[:, g, 6 * n2:6 * n2 + 6, :64],
                            ps[:, :384].rearrange("p (h c) -> p h c", c=64))

            # ================= phase 2: attention + proj =================
            with tc.tile_pool(name="attn_sb", bufs=1) as asb, \
                 tc.tile_pool(name="pT_pool", bufs=3) as ptp, \
                 tc.tile_pool(name="norm", bufs=1) as npl, \
                 tc.tile_pool(name="ysb", bufs=2) as ypl, \
                 tc.tile_pool(name="ps_sc", bufs=2, space="PSUM") as pssc, \
                 tc.tile_pool(name="ps_po", bufs=1, space="PSUM") as pspo, \
                 tc.tile_pool(name="ps_nr", bufs=1, space="PSUM") as psnr, \
                 tc.tile_pool(name="ps_dm", bufs=1, space="PSUM") as psdm:
                outT = asb.tile([128, DC, SQ], MM_DT, tag="outT")  # 24 KB
                wproj = asb.tile([128, DC, DIM], MM_DT, tag="wproj")  # 18 KB
                bias = asb.tile([128, DIM], F32, tag="bias")
                wp_v = wp_in[:].rearrange("(c p) d -> c p d", p=128)
                for dc in range(DC):
                    wps = npl.tile([128, DIM], F32, tag="wps", name=f"wps{dc}", bufs=2)
                    nc.sync.dma_start(out=wps[:], in_=wp_v[dc])
                    nc.vector.tensor_copy(wproj[:, dc, :], wps[:])
                bp_ap = bp_in[:]
                nc.gpsimd.dma_start(
                    out=bias[:],
                    in_=bass.AP(tensor=bp_ap.tensor, offset=bp_ap.offset,
                                ap=[[0, 128], [1, DIM]]))

                # half-masks for sum broadcast: hs[0, 0:128] = ones(64)+zeros,
                # hs[0, 128:256] = zeros+ones(64)  (host-provided)
                hsf = npl.tile([1, 512], F32, tag="hsf")
                nc.sync.dma_start(out=hsf[:], in_=es_in[:])
                hsel = npl.tile([1, 512], F32R, tag="hsel")
                nc.vector.tensor_copy(hsel[:], hsf[:])
                # warm-keeper: a dependency-free K=128 matmul (full array-row
                # duty) filling the PE's idle slivers while ACT runs exp, so
                # the HAM activity monitor never down-clocks the PE.
                psd = psdm.tile([128, 256], F32, tag="psd")

                def dummy_mm():
                    nc.tensor.matmul(
                        psd[:], kT[:, 0, 0:128], qT[:, 0, 0:256],
                        start=True, stop=True, skip_group_check=True)

                stages = {}
                pending_norm = []

                def emit_norm():
                    # deferred normalization of a finished head pair:
                    # broadcast the two sums rows over partition halves
                    # (rank-1 matmuls), reciprocal on all 128 DVE lanes,
                    # multiply into outT. In 512-col chunks (1 PSUM bank).
                    hp, = pending_norm
                    pending_norm.clear()
                    pdc = hp // 2
                    for n in range(SQ // 512):
                        sbc = psnr.tile([128, 512], F32, tag="sbc")
                        for i, hh in enumerate((hp - 1, hp)):
                            nc.tensor.matmul(
                                sbc[:], hsel[:, 128 * i:128 * (i + 1)],
                                stages[hh][:, n * 512:(n + 1) * 512],
                                start=(i == 0), stop=(i == 1))
                        rnorm = npl.tile([128, 512], F32, tag="rnorm")
                        nc.vector.reciprocal(rnorm[:], sbc[:])
                        nc.vector.tensor_mul(
                            outT[:, pdc, n * 512:(n + 1) * 512],
                            outT[:, pdc, n * 512:(n + 1) * 512], rnorm[:])
                    del stages[hp - 1], stages[hp]

                for h in range(HEADS):
                    dc = h // 2
                    off = 64 * (h % 2)
                    po = pspo.tile([65, SQ], F32, tag="po")
                    prev = None
                    for kc in range(TT):
                        if kc == 6 and pending_norm:
                            emit_norm()
                        pTt = ptp.tile([128, SQ], MM_DT, tag="pT")
                        ps = pssc.tile([128, SQ], F32, tag="sc")
                        for n in range(SQ // 512):
                            nc.tensor.matmul(
                                ps[:, n * 512:(n + 1) * 512],
                                kT[off:off + 64, dc, kc * 128:(kc + 1) * 128],
                                qT[off:off + 64, dc, n * 512:(n + 1) * 512],
                                start=True, stop=True)
                        nc.scalar.activation(
                            out=pTt[:], in_=ps[:],
                            func=mybir.ActivationFunctionType.Exp)
                        # attnV lags one kc behind scores so the PE never
                        # stalls waiting on the exp (keeps HAM un-throttled)
                        if prev is not None:
                            pk, pt = prev
                            for n in range(SQ // 512):
                                nc.tensor.matmul(
                                    po[:, n * 512:(n + 1) * 512],
                                    v[:, pk, VH * h:VH * h + VH],
                                    pt[:, n * 512:(n + 1) * 512],
                                    start=(pk == 0), stop=False)
                        dummy_mm()
                        prev = (kc, pTt)
                    pk, pt = prev
                    for n in range(SQ // 512):
                        nc.tensor.matmul(
                            po[:, n * 512:(n + 1) * 512],
                            v[:, pk, VH * h:VH * h + VH],
                            pt[:, n * 512:(n + 1) * 512],
                            start=False, stop=True)
                    # stash unnormalized outT (ACT, idle at head boundary)
                    # and the softmax denominators (DVE) — po frees after both
                    nc.scalar.copy(out=outT[off:off + 64, dc, :],
                                   in_=po[0:64, :])
                    stg = npl.tile([1, SQ], F32R, tag="stg", bufs=3,
                                   name=f"stg{h}")
                    nc.vector.tensor_copy(stg[:], po[64:65, :])
                    stages[h] = stg
                    if h % 2 == 1:
                        pending_norm.append(h)
                emit_norm()

                # proj: y = outT^T @ Wproj + bias
                for g0 in range(SQ // 128):
                    ys = ypl.tile([128, 1, DIM], F32, tag="ys")
                    for a in range(1):
                        t0 = (g0 + a) * 128
                        for n2 in range(2):
                            ps = pssc.tile([128, SQ], F32, tag="sc")
                            for dc in range(DC):
                                nc.tensor.matmul(
                                    ps[:, :384],
                                    outT[:, dc, t0:t0 + 128],
                                    wproj[:, dc, n2 * 384:(n2 + 1) * 384],
                                    start=(dc == 0), stop=(dc == DC - 1))
                            nc.vector.tensor_add(
                                ys[:, a, n2 * 384:(n2 + 1) * 384],
                                ps[:, :384],
                                bias[:, n2 * 384:(n2 + 1) * 384])
                    nc.sync.dma_start(out=y_v[:, g0:g0 + 1, :], in_=ys[:])

    _cap_sync_waits(nc)
    return nc


_CACHED = None


def _program():
    global _CACHED
    if _CACHED is None:
        _CACHED = _build_program()
    return _CACHED


def make_in_maps(x, Wqkv, Wproj, bproj):
    x = np.ascontiguousarray(np.asarray(x, dtype=np.float32))
    Wqkv = np.ascontiguousarray(np.asarray(Wqkv, dtype=np.float32))
    Wproj = np.ascontiguousarray(np.asarray(Wproj, dtype=np.float32))
    bproj = np.ascontiguousarray(np.asarray(bproj, dtype=np.float32))

    wq = np.ascontiguousarray(Wqkv[:, :DIM] * np.float32(SCALE))
    wk = np.ascontiguousarray(Wqkv[:, DIM:2 * DIM])
    wv = np.ascontiguousarray(Wqkv[:, 2 * DIM:])

    esel = np.zeros((1, 512), dtype=np.float32)
    esel[0, 0:64] = 1.0
    esel[0, 192:256] = 1.0

    in_maps = []
    for c in range(NCORES):
        b, q0 = c // 2, (c % 2) * SQ
        xb = np.concatenate([x[b, q0:], x[b, :q0]], axis=0)
        in_maps.append({"xb": np.ascontiguousarray(xb), "wq": wq, "wk": wk,
                        "wv": wv, "wp": Wproj, "bp": bproj, "es": esel})
    return in_maps


def kernel(x, Wqkv, Wproj, bproj):
    nc = _program()
    in_maps = make_in_maps(x, Wqkv, Wproj, bproj)
    res = run_bass_kernel_spmd(nc, in_maps, list(range(NCORES))).results
    out = np.empty((B, S, DIM), dtype=np.float32)
    for c in range(NCORES):
        b, q0 = c // 2, (c % 2) * SQ
        out[b, q0:q0 + SQ] = res[c]["y"]
    return out


# revision 36
# speedup vs baseline: 1.0103x; 1.0103x over previous
"""Trainium2 Bass kernel for 12-head attention (B=4, S=2048, D=768) on 8 cores.

Sharding: core c handles batch b=c//2, query half q0=(c%2)*1024. Each core
receives its batch's tokens rotated so its own queries are tokens 0..1023
(attention is permutation-invariant over keys, so K/V over rotated tokens
give identical output). No collectives needed.

Device algorithm (per core), everything in transposed layouts so softmax
needs no on-chip transposes:
  xT   = x^T                 [768, 2048]   (PE transpose of x tiles)
  qT   = (Wq*scale)^T @ xT   [768, 1024]   (own queries only)
  kT   = Wk^T @ xT           [768, 2048]
  V    = x @ Wv              [2048, 12*(64+1)]  (per-head 64 cols + ones col)
  per head h:
    pT[k, q]   = exp(kT_h[:,k]^T qT_h)     (no max subtraction: |scores|<~8)
    po[0:65,q] = [V_h | 1]^T @ pT          (rows 0..63 = outT, row 64 = sum)
    outT_h     = po[0:64] / po[64]         (gpsimd partition_broadcast + mul)
  y = outT^T @ Wproj + bias
All matmul operands are float32r (full fp32 storage, PE rounds to reduced
mantissa; ~1e-4 rel err, 4x faster than true fp32 matmul).
"""

import numpy as np

import concourse.bass as bass
import concourse.mybir as mybir
import concourse.tile as tile
from concourse.bass_utils import run_bass_kernel_spmd
from concourse.masks import make_identity

HEADS = 12
DIM = 768
HEAD_DIM = 64
SCALE = HEAD_DIM ** -0.5
B = 4
S = 2048
SQ = 1024          # queries per core
NCORES = 8
DC = DIM // 128    # 6 contraction chunks
TT = S // 128      # 16 token tiles
VH = HEAD_DIM + 1  # 65: per-head V cols + ones col

F32 = mybir.dt.float32
F32R = mybir.dt.float32r
BF16 = mybir.dt.bfloat16
# matmul operand dtype: float32r (safe, ~2 cyc/row) or bfloat16 (1 cyc/row)
import os
MM_DT = BF16 if os.environ.get("KMM_DT", "f32r") == "bf16" else F32R


def _cap_sync_waits(nc, max_waits=1):
    """Workaround for this walrus build's 'Too many sync wait commands':
    hoist excess per-instruction sem waits onto standalone EventSemaphore
    instructions inserted just before, on the same engine."""
    n = 0
    for fn in nc.m.functions:
        for bb in fn.blocks:
            out = []
            for inst in bb.instructions:
                si = inst.sync_info
                if si is not None and si.on_wait and len(si.on_wait) > max_waits:
                    waits = list(si.on_wait)
                    hoist, keep = waits[:-max_waits], waits[-max_waits:]
                    for w in hoist:
                        ev = mybir.InstEventSemaphore(
                            name=nc.get_next_instruction_name(), ins=[], outs=[])
                        ev.engine = inst.engine
                        ev.sync_info = mybir.SyncInfo(on_wait=[w], on_update=[])
                        out.append(ev)
                        n += 1
                    del si.on_wait[:]
                    for w in keep:
                        si.on_wait.append(w)
                out.append(inst)
            bb.instructions[:] = out
    return n




def _load_w(nc, pool, view, prefix):
    """DMA 6 [128,768] f32 weight chunks and DVE-cast them to MM_DT."""
    out = []
    for i in range(DC):
        stage = pool.tile([128, DIM], F32, tag="wstage", name=f"{prefix}s{i}",
                          bufs=2)
        nc.sync.dma_start(out=stage[:], in_=view[i])
        wt = pool.tile([128, DIM], MM_DT, tag="w", name=f"{prefix}{i}")
        nc.vector.tensor_copy(wt[:], stage[:])
        out.append(wt)
    return out

def _build_program():
    nc = bass.Bass()
    xb_in = nc.declare_dram_parameter("xb", [S, DIM], F32, isOutput=False)
    wq_in = nc.declare_dram_parameter("wq", [DIM, DIM], F32, isOutput=False)
    wk_in = nc.declare_dram_parameter("wk", [DIM, DIM], F32, isOutput=False)
    wv_in = nc.declare_dram_parameter("wv", [DIM, DIM], F32, isOutput=False)
    wp_in = nc.declare_dram_parameter("wp", [DIM, DIM], F32, isOutput=False)
    bp_in = nc.declare_dram_parameter("bp", [DIM], F32, isOutput=False)
    es_in = nc.declare_dram_parameter("es", [1, 512], F32, isOutput=False)
    y_out = nc.declare_dram_parameter("y", [SQ, DIM], F32, isOutput=True)

    xb_v = xb_in[:].rearrange("(g p) d -> p g d", p=128)   # [128, 16, 768]
    y_v = y_out[:].rearrange("(g p) d -> p g d", p=128)    # [128, 8, 768]
    w_views = {
        "wq": wq_in[:].rearrange("(c p) d -> c p d", p=128),  # [6, 128, 768]
        "wk": wk_in[:].rearrange("(c p) d -> c p d", p=128),
        "wv": wv_in[:].rearrange("(c p) d -> c p d", p=128),
    }

    with tile.TileContext(nc) as tc, \
         nc.allow_low_precision(reason="float32r matmul operands; all PSUM "
                                "accumulation stays fp32"):
        # ---- persistent tensors (live across qkv + attention) ----
        with tc.tile_pool(name="persist", bufs=1) as pp:
            kT = pp.tile([128, DC, S], MM_DT, tag="kT")       # 48 KB/part
            qT = pp.tile([128, DC, SQ], MM_DT, tag="qT")      # 24 KB/part
            v = pp.tile([128, TT, HEADS * VH], MM_DT, tag="v")  # 48.75 KB/part
            # constants: memset/affine_select can't encode f32r directly,
            # so build in f32 and round-copy via DVE.
            identf = pp.tile([128, 128], F32, tag="identf")
            make_identity(nc, identf[:])

            onesf = pp.tile([128, 64], F32, tag="onesf")
            nc.vector.memset(onesf[:], 1.0)
            # ones columns of V (col 64 of every (tile, head) group)
            v4 = v[:].rearrange("p g (h c) -> p g h c", c=VH)
            nc.vector.tensor_copy(
                v4[:, :, :, 64], onesf[:, 0:1].to_broadcast((128, TT, HEADS)))

            # ================= phase 1: xT + QKV =================
            with tc.tile_pool(name="qkv_sb", bufs=1) as qsb, \
                 tc.tile_pool(name="xnat", bufs=2) as xnp, \
                 tc.tile_pool(name="wpool", bufs=6) as wp, \
                 tc.tile_pool(name="ps_t", bufs=2, space="PSUM") as pst, \
                 tc.tile_pool(name="ps_q", bufs=3, space="PSUM") as psq:
                xT = qsb.tile([128, DC, S], MM_DT, tag="xT")  # 48 KB/part

                # transpose x into xT, 2 token-tiles per step. In bf16 mode,
                # cast during the (SWDGE) DMA so transposes run at 1 cyc/row.
                if MM_DT == BF16:
                    identt = qsb.tile([128, 128], MM_DT, tag="identt")
                    nc.vector.tensor_copy(identt[:], identf[:])
                else:
                    identt = identf
                xn_dt = MM_DT if MM_DT == BF16 else F32
                for g0 in range(0, TT, 2):
                    xn = xnp.tile([128, 2, DIM], xn_dt, tag="xn")
                    if MM_DT == BF16:
                        nc.gpsimd.dma_start(out=xn[:], in_=xb_v[:, g0:g0 + 2, :])
                    else:
                        nc.sync.dma_start(out=xn[:], in_=xb_v[:, g0:g0 + 2, :])
                    for dc in range(DC):
                        ps = pst.tile([128, 256], xn_dt, tag="pst")
                        for a in range(2):
                            nc.tensor.transpose(
                                ps[:, a * 128:(a + 1) * 128],
                                xn[:, a, dc * 128:(dc + 1) * 128],
                                identt[:])
                        nc.vector.tensor_copy(
                            xT[:, dc, g0 * 128:(g0 + 2) * 128], ps[:])

                # qT = (Wq*scale)^T @ xT  (queries = tokens 0..1023)
                wq_t = _load_w(nc, wp, w_views["wq"], "wq")
                for m in range(DC):
                    for n in range(SQ // 512):
                        ps = psq.tile([128, 512], F32, tag="psq")
                        for dc in range(DC):
                            nc.tensor.matmul(
                                ps[:],
                                wq_t[dc][:, m * 128:(m + 1) * 128],
                                xT[:, dc, n * 512:(n + 1) * 512],
                                start=(dc == 0), stop=(dc == DC - 1))
                        nc.vector.tensor_copy(qT[:, m, n * 512:(n + 1) * 512], ps[:])

                # kT = Wk^T @ xT (all tokens)
                wk_t = _load_w(nc, wp, w_views["wk"], "wk")
                for m in range(DC):
                    for n in range(S // 512):
                        ps = psq.tile([128, 512], F32, tag="psq")
                        for dc in range(DC):
                            nc.tensor.matmul(
                                ps[:],
                                wk_t[dc][:, m * 128:(m + 1) * 128],
                                xT[:, dc, n * 512:(n + 1) * 512],
                                start=(dc == 0), stop=(dc == DC - 1))
                        nc.vector.tensor_copy(kT[:, m, n * 512:(n + 1) * 512], ps[:])

                # V = x @ Wv (natural layout, strided into 65-col head groups)
                wv_t = _load_w(nc, wp, w_views["wv"], "wv")
                for g in range(TT):
                    for n2 in range(2):
                        ps = psq.tile([128, 512], F32, tag="psq")
                        for dc in range(DC):
                            nc.tensor.matmul(
                                ps[:, :384],
                                xT[:, dc, g * 128:(g + 1) * 128],
                                wv_t[dc][:, n2 * 384:(n2 + 1) * 384],
                                start=(dc == 0), stop=(dc == DC - 1))
                        nc.vector.tensor_copy(
                            v4[:, g, 6 * n2:6 * n2 + 6, :64],
                            ps[:, :384].rearrange("p (h c) -> p h c", c=64))

            # ================= phase 2: attention + proj =================
            with tc.tile_pool(name="attn_sb", bufs=1) as asb, \
                 tc.tile_pool(name="pT_pool", bufs=3) as ptp, \
                 tc.tile_pool(name="norm", bufs=1) as npl, \
                 tc.tile_pool(name="ysb", bufs=2) as ypl, \
                 tc.tile_pool(name="ps_sc", bufs=2, space="PSUM") as pssc, \
                 tc.tile_pool(name="ps_po", bufs=1, space="PSUM") as pspo, \
                 tc.tile_pool(name="ps_nr", bufs=1, space="PSUM") as psnr, \
                 tc.tile_pool(name="ps_dm", bufs=1, space="PSUM") as psdm:
                outT = asb.tile([128, DC, SQ], MM_DT, tag="outT")  # 24 KB
                wproj = asb.tile([128, DC, DIM], MM_DT, tag="wproj")  # 18 KB
                bias = asb.tile([128, DIM], F32, tag="bias")
                wp_v = wp_in[:].rearrange("(c p) d -> c p d", p=128)
                for dc in range(DC):
                    wps = npl.tile([128, DIM], F32, tag="wps", name=f"wps{dc}", bufs=2)
                    nc.sync.dma_start(out=wps[:], in_=wp_v[dc])
                    nc.vector.tensor_copy(wproj[:, dc, :], wps[:])
                bp_ap = bp_in[:]
                nc.gpsimd.dma_start(
                    out=bias[:],
                    in_=bass.AP(tensor=bp_ap.tensor, offset=bp_ap.offset,
                                ap=[[0, 128], [1, DIM]]))

                # half-masks for sum broadcast: hs[0, 0:128] = ones(64)+zeros,
                # hs[0, 128:256] = zeros+ones(64)  (host-provided)
                hsf = npl.tile([1, 512], F32, tag="hsf")
                nc.sync.dma_start(out=hsf[:], in_=es_in[:])
                hsel = npl.tile([1, 512], F32R, tag="hsel")
                nc.vector.tensor_copy(hsel[:], hsf[:])
                # warm-keeper: a dependency-free K=128 matmul (full array-row
                # duty) filling the PE's idle slivers while ACT runs exp, so
                # the HAM activity monitor never down-clocks the PE.
                psd = psdm.tile([128, 512], F32, tag="psd")

                def dummy_mm():
                    nc.tensor.matmul(
                        psd[:], kT[:, 0, 0:128], qT[:, 0, 0:512],
                        start=True, stop=True, skip_group_check=True)

                stages = {}
                pending_norm = []

                def emit_norm_half(n):
                    # deferred normalization of a finished head pair, one
                    # 512-col half at a time (spread across the next head's
                    # loop so the 1-bank sbc slot is never contended):
                    # broadcast the two sums rows over partition halves
                    # (rank-1 matmuls), reciprocal on 128 DVE lanes, multiply.
                    hp = pending_norm[0]
                    pdc = hp // 2
                    sbc = psnr.tile([128, 512], F32, tag="sbc")
                    for i, hh in enumerate((hp - 1, hp)):
                        nc.tensor.matmul(
                            sbc[:], hsel[:, 128 * i:128 * (i + 1)],
                            stages[hh][:, n * 512:(n + 1) * 512],
                            start=(i == 0), stop=(i == 1))
                    rnorm = npl.tile([128, 512], F32, tag="rnorm", bufs=2)
                    nc.vector.reciprocal(rnorm[:], sbc[:])
                    nc.vector.tensor_mul(
                        outT[:, pdc, n * 512:(n + 1) * 512],
                        outT[:, pdc, n * 512:(n + 1) * 512], rnorm[:])
                    if n == SQ // 512 - 1:
                        pending_norm.clear()
                        del stages[hp - 1], stages[hp]

                for h in range(HEADS):
                    dc = h // 2
                    off = 64 * (h % 2)
                    po = pspo.tile([65, SQ], F32, tag="po")
                    prev = None
                    for kc in range(TT):
                        if kc == 6 and pending_norm:
                            emit_norm_half(0)
                        elif kc == 11 and pending_norm:
                            emit_norm_half(1)
                        pTt = ptp.tile([128, SQ], MM_DT, tag="pT")
                        ps = pssc.tile([128, SQ], F32, tag="sc")
                        for n in range(SQ // 512):
                            nc.tensor.matmul(
                                ps[:, n * 512:(n + 1) * 512],
                                kT[off:off + 64, dc, kc * 128:(kc + 1) * 128],
                                qT[off:off + 64, dc, n * 512:(n + 1) * 512],
                                start=True, stop=True)
                        nc.scalar.activation(
                            out=pTt[:], in_=ps[:],
                            func=mybir.ActivationFunctionType.Exp)
                        # attnV lags one kc behind scores so the PE never
                        # stalls waiting on the exp (keeps HAM un-throttled)
                        if prev is not None:
                            pk, pt = prev
                            for n in range(SQ // 512):
                                nc.tensor.matmul(
                                    po[:, n * 512:(n + 1) * 512],
                                    v[:, pk, VH * h:VH * h + VH],
                                    pt[:, n * 512:(n + 1) * 512],
                                    start=(pk == 0), stop=False)
                        dummy_mm()
                        prev = (kc, pTt)
                    pk, pt = prev
                    for n in range(SQ // 512):
                        nc.tensor.matmul(
                            po[:, n * 512:(n + 1) * 512],
                            v[:, pk, VH * h:VH * h + VH],
                            pt[:, n * 512:(n + 1) * 512],
                            start=False, stop=True)
                    # stash unnormalized outT (ACT, idle at head boundary)
                    # and the softmax denominators (DVE) — po frees after both
                    nc.scalar.copy(out=outT[off:off + 64, dc, :],
                                   in_=po[0:64, :])
                    stg = npl.tile([1, SQ], F32R, tag="stg", bufs=3,
                                   name=f"stg{h}")
                    nc.vector.tensor_copy(stg[:], po[64:65, :])
                    stages[h] = stg
                    if h % 2 == 1:
                        pending_norm.append(h)
                emit_norm_half(0)
                emit_norm_half(1)

                # proj: y = outT^T @ Wproj + bias
                for g0 in range(SQ // 128):
                    ys = ypl.tile([128, 1, DIM], F32, tag="ys")
                    for a in range(1):
                        t0 = (g0 + a) * 128
                        for n2 in range(2):
                            ps = pssc.tile([128, SQ], F32, tag="sc")
                            for dc in range(DC):
                                nc.tensor.matmul(
                                    ps[:, :384],
                                    outT[:, dc, t0:t0 + 128],
                                    wproj[:, dc, n2 * 384:(n2 + 1) * 384],
                                    start=(dc == 0), stop=(dc == DC - 1))
                            nc.vector.tensor_add(
                                ys[:, a, n2 * 384:(n2 + 1) * 384],
                                ps[:, :384],
                                bias[:, n2 * 384:(n2 + 1) * 384])
                    nc.sync.dma_start(out=y_v[:, g0:g0 + 1, :], in_=ys[:])

    _cap_sync_waits(nc)
    return nc


_CACHED = None


def _program():
    global _CACHED
    if _CACHED is None:
        _CACHED = _build_program()
    return _CACHED


def make_in_maps(x, Wqkv, Wproj, bproj):
    x = np.ascontiguousarray(np.asarray(x, dtype=np.float32))
    Wqkv = np.ascontiguousarray(np.asarray(Wqkv, dtype=np.float32))
    Wproj = np.ascontiguousarray(np.asarray(Wproj, dtype=np.float32))
    bproj = np.ascontiguousarray(np.asarray(bproj, dtype=np.float32))

    wq = np.ascontiguousarray(Wqkv[:, :DIM] * np.float32(SCALE))
    wk = np.ascontiguousarray(Wqkv[:, DIM:2 * DIM])
    wv = np.ascontiguousarray(Wqkv[:, 2 * DIM:])

    esel = np.zeros((1, 512), dtype=np.float32)
    esel[0, 0:64] = 1.0
    esel[0, 192:256] = 1.0

    in_maps = []
    for c in range(NCORES):
        b, q0 = c // 2, (c % 2) * SQ
        xb = np.concatenate([x[b, q0:], x[b, :q0]], axis=0)
        in_maps.append({"xb": np.ascontiguousarray(xb), "wq": wq, "wk": wk,
                        "wv": wv, "wp": Wproj, "bp": bproj, "es": esel})
    return in_maps


def kernel(x, Wqkv, Wproj, bproj):
    nc = _program()
    in_maps = make_in_maps(x, Wqkv, Wproj, bproj)
    res = run_bass_kernel_spmd(nc, in_maps, list(range(NCORES))).results
    out = np.empty((B, S, DIM), dtype=np.float32)
    for c in range(NCORES):
        b, q0 = c // 2, (c % 2) * SQ
        out[b, q0:q0 + SQ] = res[c]["y"]
    return out


# revision 37
# speedup vs baseline: 1.1400x; 1.1283x over previous
"""Trainium2 Bass kernel for 12-head attention (B=4, S=2048, D=768) on 8 cores.

Sharding: core c handles batch b=c//2, query half q0=(c%2)*1024. Each core
receives its batch's tokens rotated so its own queries are tokens 0..1023
(attention is permutation-invariant over keys, so K/V over rotated tokens
give identical output). No collectives needed.

Device algorithm (per core), everything in transposed layouts so softmax
needs no on-chip transposes:
  xT   = x^T                 [768, 2048]   (PE transpose of x tiles)
  qT   = (Wq*scale)^T @ xT   [768, 1024]   (own queries only)
  kT   = Wk^T @ xT           [768, 2048]
  V    = x @ Wv              [2048, 12*(64+1)]  (per-head 64 cols + ones col)
  per head h:
    pT[k, q]   = exp(kT_h[:,k]^T qT_h)     (no max subtraction: |scores|<~8)
    po[0:65,q] = [V_h | 1]^T @ pT          (rows 0..63 = outT, row 64 = sum)
    outT_h     = po[0:64] / po[64]         (gpsimd partition_broadcast + mul)
  y = outT^T @ Wproj + bias
All matmul operands are float32r (full fp32 storage, PE rounds to reduced
mantissa; ~1e-4 rel err, 4x faster than true fp32 matmul).
"""

import numpy as np

import concourse.bass as bass
import concourse.mybir as mybir
import concourse.tile as tile
from concourse.bass_utils import run_bass_kernel_spmd
from concourse.masks import make_identity

HEADS = 12
DIM = 768
HEAD_DIM = 64
SCALE = HEAD_DIM ** -0.5
B = 4
S = 2048
SQ = 1024          # queries per core
NCORES = 8
DC = DIM // 128    # 6 contraction chunks
TT = S // 128      # 16 token tiles
VH = HEAD_DIM + 1  # 65: per-head V cols + ones col

F32 = mybir.dt.float32
F32R = mybir.dt.float32r
BF16 = mybir.dt.bfloat16
# matmul operand dtype: float32r (safe, ~2 cyc/row) or bfloat16 (1 cyc/row)
import os
MM_DT = BF16 if os.environ.get("KMM_DT", "f32r") == "bf16" else F32R


def _cap_sync_waits(nc, max_waits=1):
    """Workaround for this walrus build's 'Too many sync wait commands':
    hoist excess per-instruction sem waits onto standalone EventSemaphore
    instructions inserted just before, on the same engine."""
    n = 0
    for fn in nc.m.functions:
        for bb in fn.blocks:
            out = []
            for inst in bb.instructions:
                si = inst.sync_info
                if si is not None and si.on_wait and len(si.on_wait) > max_waits:
                    waits = list(si.on_wait)
                    hoist, keep = waits[:-max_waits], waits[-max_waits:]
                    for w in hoist:
                        ev = mybir.InstEventSemaphore(
                            name=nc.get_next_instruction_name(), ins=[], outs=[])
                        ev.engine = inst.engine
                        ev.sync_info = mybir.SyncInfo(on_wait=[w], on_update=[])
                        out.append(ev)
                        n += 1
                    del si.on_wait[:]
                    for w in keep:
                        si.on_wait.append(w)
                out.append(inst)
            bb.instructions[:] = out
    return n




def _load_w(nc, pool, view, prefix):
    """DMA 6 [128,768] f32 weight chunks and DVE-cast them to MM_DT."""
    out = []
    for i in range(DC):
        stage = pool.tile([128, DIM], F32, tag="wstage", name=f"{prefix}s{i}",
                          bufs=2)
        nc.sync.dma_start(out=stage[:], in_=view[i])
        wt = pool.tile([128, DIM], MM_DT, tag="w", name=f"{prefix}{i}")
        nc.vector.tensor_copy(wt[:], stage[:])
        out.append(wt)
    return out

def _build_program():
    nc = bass.Bass()
    xb_in = nc.declare_dram_parameter("xb", [S, DIM], F32, isOutput=False)
    wq_in = nc.declare_dram_parameter("wq", [DIM, DIM], F32, isOutput=False)
    wk_in = nc.declare_dram_parameter("wk", [DIM, DIM], F32, isOutput=False)
    wv_in = nc.declare_dram_parameter("wv", [DIM, DIM], F32, isOutput=False)
    wp_in = nc.declare_dram_parameter("wp", [DIM, DIM], F32, isOutput=False)
    bp_in = nc.declare_dram_parameter("bp", [DIM], F32, isOutput=False)
    es_in = nc.declare_dram_parameter("es", [1, 512], F32, isOutput=False)
    y_out = nc.declare_dram_parameter("y", [SQ, DIM], F32, isOutput=True)

    xb_v = xb_in[:].rearrange("(g p) d -> p g d", p=128)   # [128, 16, 768]
    y_v = y_out[:].rearrange("(g p) d -> p g d", p=128)    # [128, 8, 768]
    w_views = {
        "wq": wq_in[:].rearrange("(c p) d -> c p d", p=128),  # [6, 128, 768]
        "wk": wk_in[:].rearrange("(c p) d -> c p d", p=128),
        "wv": wv_in[:].rearrange("(c p) d -> c p d", p=128),
    }

    with tile.TileContext(nc) as tc, \
         nc.allow_low_precision(reason="float32r matmul operands; all PSUM "
                                "accumulation stays fp32"):
        # ---- persistent tensors (live across qkv + attention) ----
        with tc.tile_pool(name="persist", bufs=1) as pp:
            kT = pp.tile([128, DC, S], MM_DT, tag="kT")       # 48 KB/part
            qT = pp.tile([128, DC, SQ], MM_DT, tag="qT")      # 24 KB/part
            v = pp.tile([128, TT, HEADS * VH], MM_DT, tag="v")  # 48.75 KB/part
            # constants: memset/affine_select can't encode f32r directly,
            # so build in f32 and round-copy via DVE.
            identf = pp.tile([128, 128], F32, tag="identf")
            make_identity(nc, identf[:])

            onesf = pp.tile([128, 64], F32, tag="onesf")
            nc.vector.memset(onesf[:], 1.0)
            # ones columns of V (col 64 of every (tile, head) group)
            v4 = v[:].rearrange("p g (h c) -> p g h c", c=VH)
            nc.vector.tensor_copy(
                v4[:, :, :, 64], onesf[:, 0:1].to_broadcast((128, TT, HEADS)))

            # ================= phase 1: xT + QKV =================
            with tc.tile_pool(name="qkv_sb", bufs=1) as qsb, \
                 tc.tile_pool(name="xnat", bufs=2) as xnp, \
                 tc.tile_pool(name="wpool", bufs=6) as wp, \
                 tc.tile_pool(name="ps_t", bufs=2, space="PSUM") as pst, \
                 tc.tile_pool(name="ps_q", bufs=3, space="PSUM") as psq:
                xT = qsb.tile([128, DC, S], MM_DT, tag="xT")  # 48 KB/part

                # transpose x into xT, 2 token-tiles per step. In bf16 mode,
                # cast during the (SWDGE) DMA so transposes run at 1 cyc/row.
                if MM_DT == BF16:
                    identt = qsb.tile([128, 128], MM_DT, tag="identt")
                    nc.vector.tensor_copy(identt[:], identf[:])
                else:
                    identt = identf
                xn_dt = MM_DT if MM_DT == BF16 else F32
                for g0 in range(0, TT, 2):
                    xn = xnp.tile([128, 2, DIM], xn_dt, tag="xn")
                    if MM_DT == BF16:
                        nc.gpsimd.dma_start(out=xn[:], in_=xb_v[:, g0:g0 + 2, :])
                    else:
                        nc.sync.dma_start(out=xn[:], in_=xb_v[:, g0:g0 + 2, :])
                    for dc in range(DC):
                        ps = pst.tile([128, 256], xn_dt, tag="pst")
                        for a in range(2):
                            nc.tensor.transpose(
                                ps[:, a * 128:(a + 1) * 128],
                                xn[:, a, dc * 128:(dc + 1) * 128],
                                identt[:])
                        nc.vector.tensor_copy(
                            xT[:, dc, g0 * 128:(g0 + 2) * 128], ps[:])

                # qT = (Wq*scale)^T @ xT  (queries = tokens 0..1023)
                wq_t = _load_w(nc, wp, w_views["wq"], "wq")
                for m in range(DC):
                    for n in range(SQ // 512):
                        ps = psq.tile([128, 512], F32, tag="psq")
                        for dc in range(DC):
                            nc.tensor.matmul(
                                ps[:],
                                wq_t[dc][:, m * 128:(m + 1) * 128],
                                xT[:, dc, n * 512:(n + 1) * 512],
                                start=(dc == 0), stop=(dc == DC - 1))
                        nc.vector.tensor_copy(qT[:, m, n * 512:(n + 1) * 512], ps[:])

                # kT = Wk^T @ xT (all tokens)
                wk_t = _load_w(nc, wp, w_views["wk"], "wk")
                for m in range(DC):
                    for n in range(S // 512):
                        ps = psq.tile([128, 512], F32, tag="psq")
                        for dc in range(DC):
                            nc.tensor.matmul(
                                ps[:],
                                wk_t[dc][:, m * 128:(m + 1) * 128],
                                xT[:, dc, n * 512:(n + 1) * 512],
                                start=(dc == 0), stop=(dc == DC - 1))
                        nc.vector.tensor_copy(kT[:, m, n * 512:(n + 1) * 512], ps[:])

                # V = x @ Wv (natural layout, strided into 65-col head groups)
                wv_t = _load_w(nc, wp, w_views["wv"], "wv")
                for g in range(TT):
                    for n2 in range(2):
                        ps = psq.tile([128, 512], F32, tag="psq")
                        for dc in range(DC):
                            nc.tensor.matmul(
                                ps[:, :384],
                                xT[:, dc, g * 128:(g + 1) * 128],
                                wv_t[dc][:, n2 * 384:(n2 + 1) * 384],
                                start=(dc == 0), stop=(dc == DC - 1))
                        nc.vector.tensor_copy(
                            v4[:, g, 6 * n2:6 * n2 + 6, :64],
                            ps[:, :384].rearrange("p (h c) -> p h c", c=64))

            # ================= phase 2: attention + proj =================
            with tc.tile_pool(name="attn_sb", bufs=1) as asb, \
                 tc.tile_pool(name="pT_pool", bufs=3) as ptp, \
                 tc.tile_pool(name="norm", bufs=1) as npl, \
                 tc.tile_pool(name="ysb", bufs=2) as ypl, \
                 tc.tile_pool(name="ps_sc", bufs=2, space="PSUM") as pssc, \
                 tc.tile_pool(name="ps_po", bufs=1, space="PSUM") as pspo, \
                 tc.tile_pool(name="ps_nr", bufs=1, space="PSUM") as psnr, \
                 tc.tile_pool(name="ps_dm", bufs=1, space="PSUM") as psdm:
                outT = asb.tile([128, DC, SQ], MM_DT, tag="outT")  # 24 KB
                wproj = asb.tile([128, DC, DIM], MM_DT, tag="wproj")  # 18 KB
                bias = asb.tile([128, DIM], F32, tag="bias")
                wp_v = wp_in[:].rearrange("(c p) d -> c p d", p=128)
                for dc in range(DC):
                    wps = npl.tile([128, DIM], F32, tag="wps", name=f"wps{dc}", bufs=2)
                    nc.sync.dma_start(out=wps[:], in_=wp_v[dc])
                    nc.vector.tensor_copy(wproj[:, dc, :], wps[:])
                bp_ap = bp_in[:]
                nc.gpsimd.dma_start(
                    out=bias[:],
                    in_=bass.AP(tensor=bp_ap.tensor, offset=bp_ap.offset,
                                ap=[[0, 128], [1, DIM]]))

                # half-masks for sum broadcast: hs[0, 0:128] = ones(64)+zeros,
                # hs[0, 128:256] = zeros+ones(64)  (host-provided)
                hsf = npl.tile([1, 512], F32, tag="hsf")
                nc.sync.dma_start(out=hsf[:], in_=es_in[:])
                hsel = npl.tile([1, 512], F32R, tag="hsel")
                nc.vector.tensor_copy(hsel[:], hsf[:])
                # warm-keeper: a dependency-free K=128 matmul (full array-row
                # duty) filling the PE's idle slivers while ACT runs exp, so
                # the HAM activity monitor never down-clocks the PE.
                psd = psdm.tile([128, 512], F32, tag="psd")

                def dummy_mm():
                    nc.tensor.matmul(
                        psd[:], kT[:, 0, 0:128], qT[:, 0, 0:512],
                        start=True, stop=True, skip_group_check=True)

                stages = {}
                pending_norm = []

                def emit_norm_half(n):
                    # deferred normalization of a finished head pair, one
                    # 512-col half at a time (spread across the next head's
                    # loop so the 1-bank sbc slot is never contended):
                    # broadcast the two sums rows over partition halves
                    # (rank-1 matmuls), reciprocal on 128 DVE lanes, multiply.
                    hp = pending_norm[0]
                    pdc = hp // 2
                    sbc = psnr.tile([128, 512], F32, tag="sbc")
                    for i, hh in enumerate((hp - 1, hp)):
                        nc.tensor.matmul(
                            sbc[:], hsel[:, 128 * i:128 * (i + 1)],
                            stages[hh][:, n * 512:(n + 1) * 512],
                            start=(i == 0), stop=(i == 1))
                    # quick copy to SBUF so the slow reciprocal never holds
                    # the PSUM slot (which would stall the in-order PE)
                    scpy = npl.tile([128, 512], F32, tag="scpy", bufs=2)
                    nc.vector.tensor_copy(scpy[:], sbc[:])
                    rnorm = npl.tile([128, 512], F32, tag="rnorm", bufs=2)
                    nc.vector.reciprocal(rnorm[:], scpy[:])
                    nc.vector.tensor_mul(
                        outT[:, pdc, n * 512:(n + 1) * 512],
                        outT[:, pdc, n * 512:(n + 1) * 512], rnorm[:])
                    if n == SQ // 512 - 1:
                        pending_norm.clear()
                        del stages[hp - 1], stages[hp]

                for h in range(HEADS):
                    dc = h // 2
                    off = 64 * (h % 2)
                    po = pspo.tile([65, SQ], F32, tag="po")
                    prev = None
                    for kc in range(TT):
                        if kc == 6 and pending_norm:
                            emit_norm_half(0)
                        elif kc == 11 and pending_norm:
                            emit_norm_half(1)
                        pTt = ptp.tile([128, SQ], MM_DT, tag="pT")
                        ps = pssc.tile([128, SQ], F32, tag="sc")
                        for n in range(SQ // 512):
                            nc.tensor.matmul(
                                ps[:, n * 512:(n + 1) * 512],
                                kT[off:off + 64, dc, kc * 128:(kc + 1) * 128],
                                qT[off:off + 64, dc, n * 512:(n + 1) * 512],
                                start=True, stop=True)
                        nc.scalar.activation(
                            out=pTt[:], in_=ps[:],
                            func=mybir.ActivationFunctionType.Exp)
                        # attnV lags one kc behind scores so the PE never
                        # stalls waiting on the exp (keeps HAM un-throttled)
                        if prev is not None:
                            pk, pt = prev
                            for n in range(SQ // 512):
                                nc.tensor.matmul(
                                    po[:, n * 512:(n + 1) * 512],
                                    v[:, pk, VH * h:VH * h + VH],
                                    pt[:, n * 512:(n + 1) * 512],
                                    start=(pk == 0), stop=False)
                        dummy_mm()
                        prev = (kc, pTt)
                    pk, pt = prev
                    for n in range(SQ // 512):
                        nc.tensor.matmul(
                            po[:, n * 512:(n + 1) * 512],
                            v[:, pk, VH * h:VH * h + VH],
                            pt[:, n * 512:(n + 1) * 512],
                            start=False, stop=True)
                    # stash unnormalized outT (ACT, idle at head boundary)
                    # and the softmax denominators (DVE) — po frees after both
                    nc.scalar.copy(out=outT[off:off + 64, dc, :],
                                   in_=po[0:64, :])
                    stg = npl.tile([1, SQ], F32R, tag="stg", bufs=3,
                                   name=f"stg{h}")
                    nc.vector.tensor_copy(stg[:], po[64:65, :])
                    stages[h] = stg
                    if h % 2 == 1:
                        pending_norm.append(h)
                emit_norm_half(0)
                emit_norm_half(1)

                # proj: y = outT^T @ Wproj + bias
                for g0 in range(SQ // 128):
                    ys = ypl.tile([128, 1, DIM], F32, tag="ys")
                    for a in range(1):
                        t0 = (g0 + a) * 128
                        for n2 in range(2):
                            ps = pssc.tile([128, SQ], F32, tag="sc")
                            for dc in range(DC):
                                nc.tensor.matmul(
                                    ps[:, :384],
                                    outT[:, dc, t0:t0 + 128],
                                    wproj[:, dc, n2 * 384:(n2 + 1) * 384],
                                    start=(dc == 0), stop=(dc == DC - 1))
                            nc.vector.tensor_add(
                                ys[:, a, n2 * 384:(n2 + 1) * 384],
                                ps[:, :384],
                                bias[:, n2 * 384:(n2 + 1) * 384])
                    nc.sync.dma_start(out=y_v[:, g0:g0 + 1, :], in_=ys[:])

    _cap_sync_waits(nc)
    return nc


_CACHED = None


def _program():
    global _CACHED
    if _CACHED is None:
        _CACHED = _build_program()
    return _CACHED


def make_in_maps(x, Wqkv, Wproj, bproj):
    x = np.ascontiguousarray(np.asarray(x, dtype=np.float32))
    Wqkv = np.ascontiguousarray(np.asarray(Wqkv, dtype=np.float32))
    Wproj = np.ascontiguousarray(np.asarray(Wproj, dtype=np.float32))
    bproj = np.ascontiguousarray(np.asarray(bproj, dtype=np.float32))

    wq = np.ascontiguousarray(Wqkv[:, :DIM] * np.float32(SCALE))
    wk = np.ascontiguousarray(Wqkv[:, DIM:2 * DIM])
    wv = np.ascontiguousarray(Wqkv[:, 2 * DIM:])

    esel = np.zeros((1, 512), dtype=np.float32)
    esel[0, 0:64] = 1.0
    esel[0, 192:256] = 1.0

    in_maps = []
    for c in range(NCORES):
        b, q0 = c // 2, (c % 2) * SQ
        xb = np.concatenate([x[b, q0:], x[b, :q0]], axis=0)
        in_maps.append({"xb": np.ascontiguousarray(xb), "wq": wq, "wk": wk,
                        "wv": wv, "wp": Wproj, "bp": bproj, "es": esel})
    return in_maps


def kernel(x, Wqkv, Wproj, bproj):
    nc = _program()
    in_maps = make_in_maps(x, Wqkv, Wproj, bproj)
    res = run_bass_kernel_spmd(nc, in_maps, list(range(NCORES))).results
    out = np.empty((B, S, DIM), dtype=np.float32)
    for c in range(NCORES):
        b, q0 = c // 2, (c % 2) * SQ
        out[b, q0:q0 + SQ] = res[c]["y"]
    return out


# revision 38
# speedup vs baseline: 1.1476x; 1.0067x over previous
"""Trainium2 Bass kernel for 12-head attention (B=4, S=2048, D=768) on 8 cores.

Sharding: core c handles batch b=c//2, query half q0=(c%2)*1024. Each core
receives its batch's tokens rotated so its own queries are tokens 0..1023
(attention is permutation-invariant over keys, so K/V over rotated tokens
give identical output). No collectives needed.

Device algorithm (per core), everything in transposed layouts so softmax
needs no on-chip transposes:
  xT   = x^T                 [768, 2048]   (PE transpose of x tiles)
  qT   = (Wq*scale)^T @ xT   [768, 1024]   (own queries only)
  kT   = Wk^T @ xT           [768, 2048]
  V    = x @ Wv              [2048, 12*(64+1)]  (per-head 64 cols + ones col)
  per head h:
    pT[k, q]   = exp(kT_h[:,k]^T qT_h)     (no max subtraction: |scores|<~8)
    po[0:65,q] = [V_h | 1]^T @ pT          (rows 0..63 = outT, row 64 = sum)
    outT_h     = po[0:64] / po[64]         (gpsimd partition_broadcast + mul)
  y = outT^T @ Wproj + bias
All matmul operands are float32r (full fp32 storage, PE rounds to reduced
mantissa; ~1e-4 rel err, 4x faster than true fp32 matmul).
"""

import numpy as np

import concourse.bass as bass
import concourse.mybir as mybir
import concourse.tile as tile
from concourse.bass_utils import run_bass_kernel_spmd
from concourse.masks import make_identity

HEADS = 12
DIM = 768
HEAD_DIM = 64
SCALE = HEAD_DIM ** -0.5
B = 4
S = 2048
SQ = 1024          # queries per core
NCORES = 8
DC = DIM // 128    # 6 contraction chunks
TT = S // 128      # 16 token tiles
VH = HEAD_DIM + 1  # 65: per-head V cols + ones col

F32 = mybir.dt.float32
F32R = mybir.dt.float32r
BF16 = mybir.dt.bfloat16
# matmul operand dtype: float32r (safe, ~2 cyc/row) or bfloat16 (1 cyc/row)
import os
MM_DT = BF16 if os.environ.get("KMM_DT", "f32r") == "bf16" else F32R


def _cap_sync_waits(nc, max_waits=1):
    """Workaround for this walrus build's 'Too many sync wait commands':
    hoist excess per-instruction sem waits onto standalone EventSemaphore
    instructions inserted just before, on the same engine."""
    n = 0
    for fn in nc.m.functions:
        for bb in fn.blocks:
            out = []
            for inst in bb.instructions:
                si = inst.sync_info
                if si is not None and si.on_wait and len(si.on_wait) > max_waits:
                    waits = list(si.on_wait)
                    hoist, keep = waits[:-max_waits], waits[-max_waits:]
                    for w in hoist:
                        ev = mybir.InstEventSemaphore(
                            name=nc.get_next_instruction_name(), ins=[], outs=[])
                        ev.engine = inst.engine
                        ev.sync_info = mybir.SyncInfo(on_wait=[w], on_update=[])
                        out.append(ev)
                        n += 1
                    del si.on_wait[:]
                    for w in keep:
                        si.on_wait.append(w)
                out.append(inst)
            bb.instructions[:] = out
    return n




def _load_w(nc, pool, view, prefix):
    """Load 6 [128,768] weight chunks as MM_DT tiles. bf16 inputs are
    pre-cast on the host and DMA straight in; f32r stages + DVE-casts."""
    out = []
    for i in range(DC):
        wt = pool.tile([128, DIM], MM_DT, tag="w", name=f"{prefix}{i}")
        if IN_DT == MM_DT:
            nc.sync.dma_start(out=wt[:], in_=view[i])
        else:
            stage = pool.tile([128, DIM], F32, tag="wstage",
                              name=f"{prefix}s{i}", bufs=2)
            nc.sync.dma_start(out=stage[:], in_=view[i])
            nc.vector.tensor_copy(wt[:], stage[:])
        out.append(wt)
    return out

IN_DT = MM_DT if MM_DT == BF16 else F32


def _build_program():
    nc = bass.Bass()
    xb_in = nc.declare_dram_parameter("xb", [S, DIM], IN_DT, isOutput=False)
    wq_in = nc.declare_dram_parameter("wq", [DIM, DIM], IN_DT, isOutput=False)
    wk_in = nc.declare_dram_parameter("wk", [DIM, DIM], IN_DT, isOutput=False)
    wv_in = nc.declare_dram_parameter("wv", [DIM, DIM], IN_DT, isOutput=False)
    wp_in = nc.declare_dram_parameter("wp", [DIM, DIM], IN_DT, isOutput=False)
    bp_in = nc.declare_dram_parameter("bp", [DIM], F32, isOutput=False)
    es_in = nc.declare_dram_parameter("es", [1, 512], F32, isOutput=False)
    y_out = nc.declare_dram_parameter("y", [SQ, DIM], F32, isOutput=True)

    xb_v = xb_in[:].rearrange("(g p) d -> p g d", p=128)   # [128, 16, 768]
    y_v = y_out[:].rearrange("(g p) d -> p g d", p=128)    # [128, 8, 768]
    w_views = {
        "wq": wq_in[:].rearrange("(c p) d -> c p d", p=128),  # [6, 128, 768]
        "wk": wk_in[:].rearrange("(c p) d -> c p d", p=128),
        "wv": wv_in[:].rearrange("(c p) d -> c p d", p=128),
    }

    with tile.TileContext(nc) as tc, \
         nc.allow_low_precision(reason="float32r matmul operands; all PSUM "
                                "accumulation stays fp32"):
        # ---- persistent tensors (live across qkv + attention) ----
        with tc.tile_pool(name="persist", bufs=1) as pp:
            kT = pp.tile([128, DC, S], MM_DT, tag="kT")       # 48 KB/part
            qT = pp.tile([128, DC, SQ], MM_DT, tag="qT")      # 24 KB/part
            v = pp.tile([128, TT, HEADS * VH], MM_DT, tag="v")  # 48.75 KB/part
            # constants: memset/affine_select can't encode f32r directly,
            # so build in f32 and round-copy via DVE.
            identf = pp.tile([128, 128], F32, tag="identf")
            make_identity(nc, identf[:])

            onesf = pp.tile([128, 64], F32, tag="onesf")
            nc.vector.memset(onesf[:], 1.0)
            # ones columns of V (col 64 of every (tile, head) group)
            v4 = v[:].rearrange("p g (h c) -> p g h c", c=VH)
            nc.vector.tensor_copy(
                v4[:, :, :, 64], onesf[:, 0:1].to_broadcast((128, TT, HEADS)))

            # ================= phase 1: xT + QKV =================
            with tc.tile_pool(name="qkv_sb", bufs=1) as qsb, \
                 tc.tile_pool(name="xnat", bufs=2) as xnp, \
                 tc.tile_pool(name="wpool", bufs=6) as wp, \
                 tc.tile_pool(name="ps_t", bufs=2, space="PSUM") as pst, \
                 tc.tile_pool(name="ps_q", bufs=3, space="PSUM") as psq:
                xT = qsb.tile([128, DC, S], MM_DT, tag="xT")  # 48 KB/part

                # transpose x into xT, 2 token-tiles per step. In bf16 mode,
                # cast during the (SWDGE) DMA so transposes run at 1 cyc/row.
                if MM_DT == BF16:
                    identt = qsb.tile([128, 128], MM_DT, tag="identt")
                    nc.vector.tensor_copy(identt[:], identf[:])
                else:
                    identt = identf
                xn_dt = IN_DT
                for g0 in range(0, TT, 2):
                    xn = xnp.tile([128, 2, DIM], xn_dt, tag="xn")
                    nc.sync.dma_start(out=xn[:], in_=xb_v[:, g0:g0 + 2, :])
                    for dc in range(DC):
                        ps = pst.tile([128, 256], xn_dt, tag="pst")
                        for a in range(2):
                            nc.tensor.transpose(
                                ps[:, a * 128:(a + 1) * 128],
                                xn[:, a, dc * 128:(dc + 1) * 128],
                                identt[:])
                        nc.vector.tensor_copy(
                            xT[:, dc, g0 * 128:(g0 + 2) * 128], ps[:])

                # qT = (Wq*scale)^T @ xT  (queries = tokens 0..1023)
                wq_t = _load_w(nc, wp, w_views["wq"], "wq")
                for m in range(DC):
                    for n in range(SQ // 512):
                        ps = psq.tile([128, 512], F32, tag="psq")
                        for dc in range(DC):
                            nc.tensor.matmul(
                                ps[:],
                                wq_t[dc][:, m * 128:(m + 1) * 128],
                                xT[:, dc, n * 512:(n + 1) * 512],
                                start=(dc == 0), stop=(dc == DC - 1))
                        nc.vector.tensor_copy(qT[:, m, n * 512:(n + 1) * 512], ps[:])

                # kT = Wk^T @ xT (all tokens)
                wk_t = _load_w(nc, wp, w_views["wk"], "wk")
                for m in range(DC):
                    for n in range(S // 512):
                        ps = psq.tile([128, 512], F32, tag="psq")
                        for dc in range(DC):
                            nc.tensor.matmul(
                                ps[:],
                                wk_t[dc][:, m * 128:(m + 1) * 128],
                                xT[:, dc, n * 512:(n + 1) * 512],
                                start=(dc == 0), stop=(dc == DC - 1))
                        nc.vector.tensor_copy(kT[:, m, n * 512:(n + 1) * 512], ps[:])

                # V = x @ Wv (natural layout, strided into 65-col head groups)
                wv_t = _load_w(nc, wp, w_views["wv"], "wv")
                for g in range(TT):
                    for n2 in range(2):
                        ps = psq.tile([128, 512], F32, tag="psq")
                        for dc in range(DC):
                            nc.tensor.matmul(
                                ps[:, :384],
                                xT[:, dc, g * 128:(g + 1) * 128],
                                wv_t[dc][:, n2 * 384:(n2 + 1) * 384],
                                start=(dc == 0), stop=(dc == DC - 1))
                        nc.vector.tensor_copy(
                            v4[:, g, 6 * n2:6 * n2 + 6, :64],
                            ps[:, :384].rearrange("p (h c) -> p h c", c=64))

            # ================= phase 2: attention + proj =================
            with tc.tile_pool(name="attn_sb", bufs=1) as asb, \
                 tc.tile_pool(name="pT_pool", bufs=3) as ptp, \
                 tc.tile_pool(name="norm", bufs=1) as npl, \
                 tc.tile_pool(name="ysb", bufs=2) as ypl, \
                 tc.tile_pool(name="ps_sc", bufs=2, space="PSUM") as pssc, \
                 tc.tile_pool(name="ps_po", bufs=1, space="PSUM") as pspo, \
                 tc.tile_pool(name="ps_nr", bufs=1, space="PSUM") as psnr, \
                 tc.tile_pool(name="ps_dm", bufs=1, space="PSUM") as psdm:
                outT = asb.tile([128, DC, SQ], MM_DT, tag="outT")  # 24 KB
                wproj = asb.tile([128, DC, DIM], MM_DT, tag="wproj")  # 18 KB
                bias = asb.tile([128, DIM], F32, tag="bias")
                wp_v = wp_in[:].rearrange("(c p) d -> c p d", p=128)
                for dc in range(DC):
                    if IN_DT == MM_DT:
                        nc.sync.dma_start(out=wproj[:, dc, :], in_=wp_v[dc])
                    else:
                        wps = npl.tile([128, DIM], F32, tag="wps",
                                       name=f"wps{dc}", bufs=2)
                        nc.sync.dma_start(out=wps[:], in_=wp_v[dc])
                        nc.vector.tensor_copy(wproj[:, dc, :], wps[:])
                bp_ap = bp_in[:]
                nc.gpsimd.dma_start(
                    out=bias[:],
                    in_=bass.AP(tensor=bp_ap.tensor, offset=bp_ap.offset,
                                ap=[[0, 128], [1, DIM]]))

                # half-masks for sum broadcast: hs[0, 0:128] = ones(64)+zeros,
                # hs[0, 128:256] = zeros+ones(64)  (host-provided)
                hsf = npl.tile([1, 512], F32, tag="hsf")
                nc.sync.dma_start(out=hsf[:], in_=es_in[:])
                hsel = npl.tile([1, 512], F32R, tag="hsel")
                nc.vector.tensor_copy(hsel[:], hsf[:])
                # warm-keeper: a dependency-free K=128 matmul (full array-row
                # duty) filling the PE's idle slivers while ACT runs exp, so
                # the HAM activity monitor never down-clocks the PE.
                psd = psdm.tile([128, 512], F32, tag="psd")

                def dummy_mm():
                    nc.tensor.matmul(
                        psd[:], kT[:, 0, 0:128], qT[:, 0, 0:512],
                        start=True, stop=True, skip_group_check=True)

                stages = {}
                pending_norm = []

                def emit_norm_half(n):
                    # deferred normalization of a finished head pair, one
                    # 512-col half at a time (spread across the next head's
                    # loop so the 1-bank sbc slot is never contended):
                    # broadcast the two sums rows over partition halves
                    # (rank-1 matmuls), reciprocal on 128 DVE lanes, multiply.
                    hp = pending_norm[0]
                    pdc = hp // 2
                    sbc = psnr.tile([128, 512], F32, tag="sbc")
                    for i, hh in enumerate((hp - 1, hp)):
                        nc.tensor.matmul(
                            sbc[:], hsel[:, 128 * i:128 * (i + 1)],
                            stages[hh][:, n * 512:(n + 1) * 512],
                            start=(i == 0), stop=(i == 1))
                    # quick copy to SBUF so the slow reciprocal never holds
                    # the PSUM slot (which would stall the in-order PE)
                    scpy = npl.tile([128, 512], F32, tag="scpy", bufs=2)
                    nc.vector.tensor_copy(scpy[:], sbc[:])
                    rnorm = npl.tile([128, 512], F32, tag="rnorm", bufs=2)
                    nc.vector.reciprocal(rnorm[:], scpy[:])
                    nc.vector.tensor_mul(
                        outT[:, pdc, n * 512:(n + 1) * 512],
                        outT[:, pdc, n * 512:(n + 1) * 512], rnorm[:])
                    if n == SQ // 512 - 1:
                        pending_norm.clear()
                        del stages[hp - 1], stages[hp]

                for h in range(HEADS):
                    dc = h // 2
                    off = 64 * (h % 2)
                    po = pspo.tile([65, SQ], F32, tag="po")
                    prev = None
                    for kc in range(TT):
                        if kc == 6 and pending_norm:
                            emit_norm_half(0)
                        elif kc == 11 and pending_norm:
                            emit_norm_half(1)
                        pTt = ptp.tile([128, SQ], MM_DT, tag="pT")
                        ps = pssc.tile([128, SQ], F32, tag="sc")
                        for n in range(SQ // 512):
                            nc.tensor.matmul(
                                ps[:, n * 512:(n + 1) * 512],
                                kT[off:off + 64, dc, kc * 128:(kc + 1) * 128],
                                qT[off:off + 64, dc, n * 512:(n + 1) * 512],
                                start=True, stop=True)
                        nc.scalar.activation(
                            out=pTt[:], in_=ps[:],
                            func=mybir.ActivationFunctionType.Exp)
                        # attnV lags one kc behind scores so the PE never
                        # stalls waiting on the exp (keeps HAM un-throttled)
                        if prev is not None:
                            pk, pt = prev
                            for n in range(SQ // 512):
                                nc.tensor.matmul(
                                    po[:, n * 512:(n + 1) * 512],
                                    v[:, pk, VH * h:VH * h + VH],
                                    pt[:, n * 512:(n + 1) * 512],
                                    start=(pk == 0), stop=False)
                        dummy_mm()
                        prev = (kc, pTt)
                    pk, pt = prev
                    for n in range(SQ // 512):
                        nc.tensor.matmul(
                            po[:, n * 512:(n + 1) * 512],
                            v[:, pk, VH * h:VH * h + VH],
                            pt[:, n * 512:(n + 1) * 512],
                            start=False, stop=True)
                    # stash unnormalized outT (ACT, idle at head boundary)
                    # and the softmax denominators (DVE) — po frees after both
                    nc.scalar.copy(out=outT[off:off + 64, dc, :],
                                   in_=po[0:64, :])
                    stg = npl.tile([1, SQ], F32R, tag="stg", bufs=3,
                                   name=f"stg{h}")
                    nc.vector.tensor_copy(stg[:], po[64:65, :])
                    stages[h] = stg
                    if h % 2 == 1:
                        pending_norm.append(h)
                emit_norm_half(0)
                emit_norm_half(1)

                # proj: y = outT^T @ Wproj + bias
                for g0 in range(SQ // 128):
                    ys = ypl.tile([128, 1, DIM], F32, tag="ys")
                    for a in range(1):
                        t0 = (g0 + a) * 128
                        for n2 in range(2):
                            ps = pssc.tile([128, SQ], F32, tag="sc")
                            for dc in range(DC):
                                nc.tensor.matmul(
                                    ps[:, :384],
                                    outT[:, dc, t0:t0 + 128],
                                    wproj[:, dc, n2 * 384:(n2 + 1) * 384],
                                    start=(dc == 0), stop=(dc == DC - 1))
                            nc.vector.tensor_add(
                                ys[:, a, n2 * 384:(n2 + 1) * 384],
                                ps[:, :384],
                                bias[:, n2 * 384:(n2 + 1) * 384])
                    nc.sync.dma_start(out=y_v[:, g0:g0 + 1, :], in_=ys[:])

    _cap_sync_waits(nc)
    return nc


_CACHED = None


def _program():
    global _CACHED
    if _CACHED is None:
        _CACHED = _build_program()
    return _CACHED


def make_in_maps(x, Wqkv, Wproj, bproj):
    import ml_dtypes
    in_np = ml_dtypes.bfloat16 if MM_DT == BF16 else np.float32

    x = np.ascontiguousarray(np.asarray(x, dtype=np.float32))
    Wqkv = np.ascontiguousarray(np.asarray(Wqkv, dtype=np.float32))
    Wproj = np.ascontiguousarray(np.asarray(Wproj, dtype=np.float32)
                                 .astype(in_np))
    bproj = np.ascontiguousarray(np.asarray(bproj, dtype=np.float32))

    wq = np.ascontiguousarray((Wqkv[:, :DIM] * np.float32(SCALE))
                              .astype(in_np))
    wk = np.ascontiguousarray(Wqkv[:, DIM:2 * DIM].astype(in_np))
    wv = np.ascontiguousarray(Wqkv[:, 2 * DIM:].astype(in_np))

    esel = np.zeros((1, 512), dtype=np.float32)
    esel[0, 0:64] = 1.0
    esel[0, 192:256] = 1.0

    in_maps = []
    for c in range(NCORES):
        b, q0 = c // 2, (c % 2) * SQ
        xb = np.concatenate([x[b, q0:], x[b, :q0]], axis=0).astype(in_np)
        in_maps.append({"xb": np.ascontiguousarray(xb), "wq": wq, "wk": wk,
                        "wv": wv, "wp": Wproj, "bp": bproj, "es": esel})
    return in_maps


def kernel(x, Wqkv, Wproj, bproj):
    nc = _program()
    in_maps = make_in_maps(x, Wqkv, Wproj, bproj)
    res = run_bass_kernel_spmd(nc, in_maps, list(range(NCORES))).results
    out = np.empty((B, S, DIM), dtype=np.float32)
    for c in range(NCORES):
        b, q0 = c // 2, (c % 2) * SQ
        out[b, q0:q0 + SQ] = res[c]["y"]
    return out


# revision 39
# speedup vs baseline: 1.1856x; 1.0331x over previous
"""Trainium2 Bass kernel for 12-head attention (B=4, S=2048, D=768) on 8 cores.

Sharding: core c handles batch b=c//2, query half q0=(c%2)*1024. Each core
receives its batch's tokens rotated so its own queries are tokens 0..1023
(attention is permutation-invariant over keys, so K/V over rotated tokens
give identical output). No collectives needed.

Device algorithm (per core), everything in transposed layouts so softmax
needs no on-chip transposes:
  xT   = x^T                 [768, 2048]   (PE transpose of x tiles)
  qT   = (Wq*scale)^T @ xT   [768, 1024]   (own queries only)
  kT   = Wk^T @ xT           [768, 2048]
  V    = x @ Wv              [2048, 12*(64+1)]  (per-head 64 cols + ones col)
  per head h:
    pT[k, q]   = exp(kT_h[:,k]^T qT_h)     (no max subtraction: |scores|<~8)
    po[0:65,q] = [V_h | 1]^T @ pT          (rows 0..63 = outT, row 64 = sum)
    outT_h     = po[0:64] / po[64]         (gpsimd partition_broadcast + mul)
  y = outT^T @ Wproj + bias
All matmul operands are float32r (full fp32 storage, PE rounds to reduced
mantissa; ~1e-4 rel err, 4x faster than true fp32 matmul).
"""

import numpy as np

import concourse.bass as bass
import concourse.mybir as mybir
import concourse.tile as tile
from concourse.bass_utils import run_bass_kernel_spmd
from concourse.masks import make_identity

HEADS = 12
DIM = 768
HEAD_DIM = 64
SCALE = HEAD_DIM ** -0.5
B = 4
S = 2048
SQ = 1024          # queries per core
NCORES = 8
DC = DIM // 128    # 6 contraction chunks
TT = S // 128      # 16 token tiles
VH = HEAD_DIM + 1  # 65: per-head V cols + ones col

F32 = mybir.dt.float32
F32R = mybir.dt.float32r
BF16 = mybir.dt.bfloat16
# matmul operand dtype: float32r (safe, ~2 cyc/row) or bfloat16 (1 cyc/row)
import os
MM_DT = BF16 if os.environ.get("KMM_DT", "f32r") == "bf16" else F32R


def _cap_sync_waits(nc, max_waits=1):
    """Workaround for this walrus build's 'Too many sync wait commands':
    hoist excess per-instruction sem waits onto standalone EventSemaphore
    instructions inserted just before, on the same engine."""
    n = 0
    for fn in nc.m.functions:
        for bb in fn.blocks:
            out = []
            for inst in bb.instructions:
                si = inst.sync_info
                if si is not None and si.on_wait and len(si.on_wait) > max_waits:
                    waits = list(si.on_wait)
                    hoist, keep = waits[:-max_waits], waits[-max_waits:]
                    for w in hoist:
                        ev = mybir.InstEventSemaphore(
                            name=nc.get_next_instruction_name(), ins=[], outs=[])
                        ev.engine = inst.engine
                        ev.sync_info = mybir.SyncInfo(on_wait=[w], on_update=[])
                        out.append(ev)
                        n += 1
                    del si.on_wait[:]
                    for w in keep:
                        si.on_wait.append(w)
                out.append(inst)
            bb.instructions[:] = out
    return n




def _load_w(nc, pool, view, prefix):
    """Load 6 [128,768] weight chunks as MM_DT tiles. bf16 inputs are
    pre-cast on the host and DMA straight in; f32r stages + DVE-casts."""
    out = []
    for i in range(DC):
        wt = pool.tile([128, DIM], MM_DT, tag="w", name=f"{prefix}{i}")
        if IN_DT == MM_DT:
            nc.sync.dma_start(out=wt[:], in_=view[i])
        else:
            stage = pool.tile([128, DIM], F32, tag="wstage",
                              name=f"{prefix}s{i}", bufs=2)
            nc.sync.dma_start(out=stage[:], in_=view[i])
            nc.vector.tensor_copy(wt[:], stage[:])
        out.append(wt)
    return out

IN_DT = MM_DT if MM_DT == BF16 else F32


def _build_program():
    nc = bass.Bass()
    xb_in = nc.declare_dram_parameter("xb", [S, DIM], IN_DT, isOutput=False)
    wq_in = nc.declare_dram_parameter("wq", [DIM, DIM], IN_DT, isOutput=False)
    wk_in = nc.declare_dram_parameter("wk", [DIM, DIM], IN_DT, isOutput=False)
    wv_in = nc.declare_dram_parameter("wv", [DIM, DIM], IN_DT, isOutput=False)
    wp_in = nc.declare_dram_parameter("wp", [DIM, DIM], IN_DT, isOutput=False)
    bp_in = nc.declare_dram_parameter("bp", [DIM], F32, isOutput=False)
    es_in = nc.declare_dram_parameter("es", [1, 512], F32, isOutput=False)
    y_out = nc.declare_dram_parameter("y", [SQ, DIM], F32, isOutput=True)

    xb_v = xb_in[:].rearrange("(g p) d -> p g d", p=128)   # [128, 16, 768]
    y_v = y_out[:].rearrange("(g p) d -> p g d", p=128)    # [128, 8, 768]
    w_views = {
        "wq": wq_in[:].rearrange("(c p) d -> c p d", p=128),  # [6, 128, 768]
        "wk": wk_in[:].rearrange("(c p) d -> c p d", p=128),
        "wv": wv_in[:].rearrange("(c p) d -> c p d", p=128),
    }

    with tile.TileContext(nc) as tc, \
         nc.allow_low_precision(reason="float32r matmul operands; all PSUM "
                                "accumulation stays fp32"):
        # ---- persistent tensors (live across qkv + attention) ----
        with tc.tile_pool(name="persist", bufs=1) as pp:
            kT = pp.tile([128, DC, S], MM_DT, tag="kT")       # 48 KB/part
            qT = pp.tile([128, DC, SQ], MM_DT, tag="qT")      # 24 KB/part
            v = pp.tile([128, TT, HEADS * VH], MM_DT, tag="v")  # 48.75 KB/part
            # constants: memset/affine_select can't encode f32r directly,
            # so build in f32 and round-copy via DVE.
            identf = pp.tile([128, 128], F32, tag="identf")
            make_identity(nc, identf[:])

            onesf = pp.tile([128, 64], F32, tag="onesf")
            nc.vector.memset(onesf[:], 1.0)
            # ones columns of V (col 64 of every (tile, head) group)
            v4 = v[:].rearrange("p g (h c) -> p g h c", c=VH)
            nc.vector.tensor_copy(
                v4[:, :, :, 64], onesf[:, 0:1].to_broadcast((128, TT, HEADS)))

            # ================= phase 1: xT + QKV =================
            with tc.tile_pool(name="qkv_sb", bufs=1) as qsb, \
                 tc.tile_pool(name="xnat", bufs=2) as xnp, \
                 tc.tile_pool(name="wpool", bufs=6) as wp, \
                 tc.tile_pool(name="ps_t", bufs=2, space="PSUM") as pst, \
                 tc.tile_pool(name="ps_q", bufs=3, space="PSUM") as psq:
                xT = qsb.tile([128, DC, S], MM_DT, tag="xT")  # 48 KB/part

                # transpose x into xT, 2 token-tiles per step. In bf16 mode,
                # cast during the (SWDGE) DMA so transposes run at 1 cyc/row.
                if MM_DT == BF16:
                    identt = qsb.tile([128, 128], MM_DT, tag="identt")
                    nc.vector.tensor_copy(identt[:], identf[:])
                else:
                    identt = identf
                xn_dt = IN_DT
                for g0 in range(0, TT, 2):
                    xn = xnp.tile([128, 2, DIM], xn_dt, tag="xn")
                    if g0 == 0:
                        # split the first load so transposes start sooner
                        nc.sync.dma_start(out=xn[:, 0, :], in_=xb_v[:, 0, :])
                        nc.sync.dma_start(out=xn[:, 1, :], in_=xb_v[:, 1, :])
                    else:
                        nc.sync.dma_start(out=xn[:], in_=xb_v[:, g0:g0 + 2, :])
                    for dc in range(DC):
                        ps = pst.tile([128, 256], xn_dt, tag="pst")
                        for a in range(2):
                            nc.tensor.transpose(
                                ps[:, a * 128:(a + 1) * 128],
                                xn[:, a, dc * 128:(dc + 1) * 128],
                                identt[:])
                        nc.vector.tensor_copy(
                            xT[:, dc, g0 * 128:(g0 + 2) * 128], ps[:])

                # qT = (Wq*scale)^T @ xT  (queries = tokens 0..1023)
                wq_t = _load_w(nc, wp, w_views["wq"], "wq")
                for m in range(DC):
                    for n in range(SQ // 512):
                        ps = psq.tile([128, 512], F32, tag="psq")
                        for dc in range(DC):
                            nc.tensor.matmul(
                                ps[:],
                                wq_t[dc][:, m * 128:(m + 1) * 128],
                                xT[:, dc, n * 512:(n + 1) * 512],
                                start=(dc == 0), stop=(dc == DC - 1))
                        nc.vector.tensor_copy(qT[:, m, n * 512:(n + 1) * 512], ps[:])

                # kT = Wk^T @ xT (all tokens)
                wk_t = _load_w(nc, wp, w_views["wk"], "wk")
                for m in range(DC):
                    for n in range(S // 512):
                        ps = psq.tile([128, 512], F32, tag="psq")
                        for dc in range(DC):
                            nc.tensor.matmul(
                                ps[:],
                                wk_t[dc][:, m * 128:(m + 1) * 128],
                                xT[:, dc, n * 512:(n + 1) * 512],
                                start=(dc == 0), stop=(dc == DC - 1))
                        nc.vector.tensor_copy(kT[:, m, n * 512:(n + 1) * 512], ps[:])

                # V = x @ Wv (natural layout, strided into 65-col head groups)
                wv_t = _load_w(nc, wp, w_views["wv"], "wv")
                for g in range(TT):
                    for n2 in range(2):
                        ps = psq.tile([128, 512], F32, tag="psq")
                        for dc in range(DC):
                            nc.tensor.matmul(
                                ps[:, :384],
                                xT[:, dc, g * 128:(g + 1) * 128],
                                wv_t[dc][:, n2 * 384:(n2 + 1) * 384],
                                start=(dc == 0), stop=(dc == DC - 1))
                        nc.vector.tensor_copy(
                            v4[:, g, 6 * n2:6 * n2 + 6, :64],
                            ps[:, :384].rearrange("p (h c) -> p h c", c=64))

            # ================= phase 2: attention + proj =================
            with tc.tile_pool(name="attn_sb", bufs=1) as asb, \
                 tc.tile_pool(name="pT_pool", bufs=3) as ptp, \
                 tc.tile_pool(name="norm", bufs=1) as npl, \
                 tc.tile_pool(name="ysb", bufs=2) as ypl, \
                 tc.tile_pool(name="ps_sc", bufs=2, space="PSUM") as pssc, \
                 tc.tile_pool(name="ps_po", bufs=1, space="PSUM") as pspo, \
                 tc.tile_pool(name="ps_nr", bufs=1, space="PSUM") as psnr, \
                 tc.tile_pool(name="ps_dm", bufs=1, space="PSUM") as psdm:
                outT = asb.tile([128, DC, SQ], MM_DT, tag="outT")  # 24 KB
                wproj = asb.tile([128, DC, DIM], MM_DT, tag="wproj")  # 18 KB
                bias = asb.tile([128, DIM], F32, tag="bias")
                wp_v = wp_in[:].rearrange("(c p) d -> c p d", p=128)
                for dc in range(DC):
                    if IN_DT == MM_DT:
                        nc.sync.dma_start(out=wproj[:, dc, :], in_=wp_v[dc])
                    else:
                        wps = npl.tile([128, DIM], F32, tag="wps",
                                       name=f"wps{dc}", bufs=2)
                        nc.sync.dma_start(out=wps[:], in_=wp_v[dc])
                        nc.vector.tensor_copy(wproj[:, dc, :], wps[:])
                bp_ap = bp_in[:]
                nc.gpsimd.dma_start(
                    out=bias[:],
                    in_=bass.AP(tensor=bp_ap.tensor, offset=bp_ap.offset,
                                ap=[[0, 128], [1, DIM]]))

                # half-masks for sum broadcast: hs[0, 0:128] = ones(64)+zeros,
                # hs[0, 128:256] = zeros+ones(64)  (host-provided)
                hsf = npl.tile([1, 512], F32, tag="hsf")
                nc.sync.dma_start(out=hsf[:], in_=es_in[:])
                hsel = npl.tile([1, 512], F32R, tag="hsel")
                nc.vector.tensor_copy(hsel[:], hsf[:])
                # warm-keeper: a dependency-free K=128 matmul (full array-row
                # duty) filling the PE's idle slivers while ACT runs exp, so
                # the HAM activity monitor never down-clocks the PE.
                psd = psdm.tile([128, 512], F32, tag="psd")

                def dummy_mm():
                    nc.tensor.matmul(
                        psd[:], kT[:, 0, 0:128], qT[:, 0, 0:512],
                        start=True, stop=True, skip_group_check=True)

                stages = {}
                pending_norm = []

                def emit_norm_half(n):
                    # deferred normalization of a finished head pair, one
                    # 512-col half at a time (spread across the next head's
                    # loop so the 1-bank sbc slot is never contended):
                    # broadcast the two sums rows over partition halves
                    # (rank-1 matmuls), reciprocal on 128 DVE lanes, multiply.
                    hp = pending_norm[0]
                    pdc = hp // 2
                    sbc = psnr.tile([128, 512], F32, tag="sbc")
                    for i, hh in enumerate((hp - 1, hp)):
                        nc.tensor.matmul(
                            sbc[:], hsel[:, 128 * i:128 * (i + 1)],
                            stages[hh][:, n * 512:(n + 1) * 512],
                            start=(i == 0), stop=(i == 1))
                    # quick copy to SBUF so the slow reciprocal never holds
                    # the PSUM slot (which would stall the in-order PE)
                    scpy = npl.tile([128, 512], F32, tag="scpy", bufs=2)
                    nc.vector.tensor_copy(scpy[:], sbc[:])
                    rnorm = npl.tile([128, 512], F32, tag="rnorm", bufs=2)
                    nc.vector.reciprocal(rnorm[:], scpy[:])
                    nc.vector.tensor_mul(
                        outT[:, pdc, n * 512:(n + 1) * 512],
                        outT[:, pdc, n * 512:(n + 1) * 512], rnorm[:])
                    if n == SQ // 512 - 1:
                        pending_norm.clear()
                        del stages[hp - 1], stages[hp]

                for h in range(HEADS):
                    dc = h // 2
                    off = 64 * (h % 2)
                    po = pspo.tile([65, SQ], F32, tag="po")
                    prev = None
                    for kc in range(TT):
                        if kc == 6 and pending_norm:
                            emit_norm_half(0)
                        elif kc == 11 and pending_norm:
                            emit_norm_half(1)
                        pTt = ptp.tile([128, SQ], MM_DT, tag="pT", bufs=4)
                        ps = pssc.tile([128, SQ], F32, tag="sc")
                        for n in range(SQ // 512):
                            nc.tensor.matmul(
                                ps[:, n * 512:(n + 1) * 512],
                                kT[off:off + 64, dc, kc * 128:(kc + 1) * 128],
                                qT[off:off + 64, dc, n * 512:(n + 1) * 512],
                                start=True, stop=True)
                        nc.scalar.activation(
                            out=pTt[:], in_=ps[:],
                            func=mybir.ActivationFunctionType.Exp)
                        # attnV lags one kc behind scores so the PE never
                        # stalls waiting on the exp (keeps HAM un-throttled)
                        if prev is not None:
                            pk, pt = prev
                            for n in range(SQ // 512):
                                nc.tensor.matmul(
                                    po[:, n * 512:(n + 1) * 512],
                                    v[:, pk, VH * h:VH * h + VH],
                                    pt[:, n * 512:(n + 1) * 512],
                                    start=(pk == 0), stop=False)
                        dummy_mm()
                        prev = (kc, pTt)
                    pk, pt = prev
                    for n in range(SQ // 512):
                        nc.tensor.matmul(
                            po[:, n * 512:(n + 1) * 512],
                            v[:, pk, VH * h:VH * h + VH],
                            pt[:, n * 512:(n + 1) * 512],
                            start=False, stop=True)
                    # stash unnormalized outT (ACT, idle at head boundary)
                    # and the softmax denominators (DVE) — po frees after both
                    nc.scalar.copy(out=outT[off:off + 64, dc, :],
                                   in_=po[0:64, :])
                    stg = npl.tile([1, SQ], F32R, tag="stg", bufs=3,
                                   name=f"stg{h}")
                    nc.vector.tensor_copy(stg[:], po[64:65, :])
                    stages[h] = stg
                    if h % 2 == 1:
                        pending_norm.append(h)
                emit_norm_half(0)
                emit_norm_half(1)

                # proj: y = outT^T @ Wproj + bias
                for g0 in range(SQ // 128):
                    ys = ypl.tile([128, 1, DIM], F32, tag="ys")
                    for a in range(1):
                        t0 = (g0 + a) * 128
                        for n2 in range(2):
                            ps = pssc.tile([128, SQ], F32, tag="sc")
                            for dc in range(DC):
                                nc.tensor.matmul(
                                    ps[:, :384],
                                    outT[:, dc, t0:t0 + 128],
                                    wproj[:, dc, n2 * 384:(n2 + 1) * 384],
                                    start=(dc == 0), stop=(dc == DC - 1))
                            nc.vector.tensor_add(
                                ys[:, a, n2 * 384:(n2 + 1) * 384],
                                ps[:, :384],
                                bias[:, n2 * 384:(n2 + 1) * 384])
                    nc.sync.dma_start(out=y_v[:, g0:g0 + 1, :], in_=ys[:])

    _cap_sync_waits(nc)
    return nc


_CACHED = None


def _program():
    global _CACHED
    if _CACHED is None:
        _CACHED = _build_program()
    return _CACHED


def make_in_maps(x, Wqkv, Wproj, bproj):
    import ml_dtypes
    in_np = ml_dtypes.bfloat16 if MM_DT == BF16 else np.float32

    x = np.ascontiguousarray(np.asarray(x, dtype=np.float32))
    Wqkv = np.ascontiguousarray(np.asarray(Wqkv, dtype=np.float32))
    Wproj = np.ascontiguousarray(np.asarray(Wproj, dtype=np.float32)
                                 .astype(in_np))
    bproj = np.ascontiguousarray(np.asarray(bproj, dtype=np.float32))

    wq = np.ascontiguousarray((Wqkv[:, :DIM] * np.float32(SCALE))
                              .astype(in_np))
    wk = np.ascontiguousarray(Wqkv[:, DIM:2 * DIM].astype(in_np))
    wv = np.ascontiguousarray(Wqkv[:, 2 * DIM:].astype(in_np))

    esel = np.zeros((1, 512), dtype=np.float32)
    esel[0, 0:64] = 1.0
    esel[0, 192:256] = 1.0

    in_maps = []
    for c in range(NCORES):
        b, q0 = c // 2, (c % 2) * SQ
        xb = np.concatenate([x[b, q0:], x[b, :q0]], axis=0).astype(in_np)
        in_maps.append({"xb": np.ascontiguousarray(xb), "wq": wq, "wk": wk,
                        "wv": wv, "wp": Wproj, "bp": bproj, "es": esel})
    return in_maps


def kernel(x, Wqkv, Wproj, bproj):
    nc = _program()
    in_maps = make_in_maps(x, Wqkv, Wproj, bproj)
    res = run_bass_kernel_spmd(nc, in_maps, list(range(NCORES))).results
    out = np.empty((B, S, DIM), dtype=np.float32)
    for c in range(NCORES):
        b, q0 = c // 2, (c % 2) * SQ
        out[b, q0:q0 + SQ] = res[c]["y"]
    return out
